# revision 7
# baseline (speedup 1.0000x reference)
"""Trainium2 Bass kernel for nn_DCNConv (conv+BN+SiLU -> offset conv -> 3x3
deformable conv -> BN+SiLU), data-parallel over batch across 8 NeuronCores.

Self-contained: hardcodes all shapes. The deformable bilinear sampling is
computed exactly via "tent" weights: for offsets |off| < 1 the bilinear
interpolation at p + off equals sum_{d in {-1,0,1}} relu(1-|off-d|) * h[p+d]
(this reproduces the floor/frac 2-corner form including the off<0 branch and
matches zero-padded sampling when h is zero-padded). The per-tap 2D sample is
the separable product over y/x, i.e. 9 shifted elementwise MACs on the DVE.
"""

import numpy as np
import ml_dtypes

import concourse.bass as bass
import concourse.mybir as mybir
import concourse.tile as tile
from concourse.vector_clock import ScopedClock

# ---------------------------------------------------------------------------
# Workaround: this container's walrus build rejects >1 sem-wait on an SP CTRL
# instruction ("Too many sync wait commands") which the stock TileContext tail
# drain hits. Split the tail-drain waits across single-wait SP nops.
_MAX_WAITS = 1


def _patched_drain_and_barrier(self, tick_clock, wait_clock):
    nc = self.nc
    n1 = nc.sync.nop()
    wait_clock.add_sem_waits(n1.ins, ScopedClock({None: tick_clock.global_clock}))
    waits = list(n1.ins.sync_info.on_wait)
    if len(waits) > _MAX_WAITS:
        n1.ins.sync_info.on_wait = waits[:_MAX_WAITS]
        for i in range(_MAX_WAITS, len(waits), _MAX_WAITS):
            n = nc.sync.nop()
            n.ins.sync_info = mybir.SyncInfo(
                on_wait=waits[i : i + _MAX_WAITS], on_update=[]
            )
    nc.sync.drain()
    nc.all_engine_barrier()
    assert self.sems is not None
    popped = nc._tile_sem_poison_stack.pop()
    assert popped is self._sem_poison
    nc.clear_and_free_semaphores(list(self.sems.allocated().values()))
    nc.all_engine_barrier()


tile.TileContext._drain_and_barrier = _patched_drain_and_barrier

# ---------------------------------------------------------------------------

EPS = 1e-5
B, C1, C2, H, W = 8, 128, 256, 160, 160
OH = OW = 80
P = OH * OW  # 6400 output positions
XR = 161  # padded input rows (1 top pad)
XCE = 81  # even padded cols 0,2,...,160
XCO = 80  # odd padded cols 1,3,...,159
HP = 84  # padded h spatial (2-border)
N_CORES = 8

SLAB_ROWS = 16
SL = SLAB_ROWS * OW  # 1280
N_SLABS = OH // SLAB_ROWS  # 5
N_MM = 320  # dconv matmul moving size (4 per slab)
CONV_ROWS = 5  # conv1/offset rows per matmul tile (N=400)
N_CT = OH // CONV_ROWS  # 16 tiles

f32 = mybir.dt.float32
f32r = mybir.dt.float32r
bf16 = mybir.dt.bfloat16


def _ap(t_ap, offset, dims):
    """Raw AP over a tile's underlying tensor. dims: [[stride, count], ...]
    in elements, first entry is the partition dim."""
    return bass.AP(tensor=t_ap.tensor, offset=t_ap.offset + offset, ap=dims)


def _build(debug=False):
    nc = bass.Bass()

    # DRAM I/O (per-core)
    xe_d = nc.dram_tensor("xe", (C1, XR, XCE), f32r, kind="ExternalInput")
    xo_d = nc.dram_tensor("xo", (C1, XR, XCO), f32r, kind="ExternalInput")
    w1t_d = nc.dram_tensor("w1t", (C1, 9, C2), f32r, kind="ExternalInput")
    offt_d = nc.dram_tensor("offt", (128, 9, 2, 18), bf16, kind="ExternalInput")
    dwt_d = nc.dram_tensor("dwt", (128, 9, 2, C2), bf16, kind="ExternalInput")
    t1_d = nc.dram_tensor("t1", (128, 2), f32, kind="ExternalInput")
    t2_d = nc.dram_tensor("t2", (128, 2), f32, kind="ExternalInput")
    offb_d = nc.dram_tensor("offb", (18, 1), f32, kind="ExternalInput")
    buv_d = nc.dram_tensor("buv", (81, 2), f32, kind="ExternalInput")
    out_d = nc.dram_tensor("out", (C2, P), f32, kind="ExternalOutput")
    offst_d = nc.dram_tensor("offst", (18, P), f32)  # internal staging
    tbf_d = nc.dram_tensor("tbfst", (81, P), bf16)  # internal staging
    if debug:
        hdbg_d = nc.dram_tensor("hdbg", (2, 128, HP * HP), f32, kind="ExternalOutput")
        offdbg_d = nc.dram_tensor("offdbg", (18, P), f32, kind="ExternalOutput")

    with tile.TileContext(nc) as tc:
        with (
            tc.tile_pool(name="const", bufs=1) as cpool,
            tc.tile_pool(name="hpool", bufs=1) as hpool,
        ):
            w1t = cpool.tile([C1, 9, C2], f32r)
            nc.sync.dma_start(out=w1t[:], in_=w1t_d[:])
            offt = cpool.tile([128, 9, 2, 18], bf16)
            nc.sync.dma_start(out=offt[:], in_=offt_d[:])
            dwt = cpool.tile([128, 9, 2, C2], bf16)
            nc.sync.dma_start(out=dwt[:], in_=dwt_d[:])
            t1 = cpool.tile([128, 2], f32)
            nc.sync.dma_start(out=t1[:], in_=t1_d[:])
            t2 = cpool.tile([128, 2], f32)
            nc.sync.dma_start(out=t2[:], in_=t2_d[:])
            offb = cpool.tile([18, 1], f32)
            nc.sync.dma_start(out=offb[:], in_=offb_d[:])
            buv = cpool.tile([81, 2], f32)
            nc.sync.dma_start(out=buv[:], in_=buv_d[:])

            # padded h (bf16), interior at [2:82, 2:82]
            h_pad = [hpool.tile([128, HP, HP], bf16, name=f"h_pad{i}") for i in range(2)]
            for cc in range(2):
                nc.vector.memset(h_pad[cc][:], 0.0)

            # ---------------- Stage 1: conv1 + BN1 + SiLU -> h ----------------
            with (
                tc.tile_pool(name="xp", bufs=1) as xpool,
                tc.tile_pool(name="ps1", bufs=4, space="PSUM") as ps1pool,
            ):
                xe = xpool.tile([C1, XR, XCE], f32r)
                nc.sync.dma_start(out=xe[:], in_=xe_d[:])
                xo = xpool.tile([C1, XR, XCO], f32r)
                nc.sync.dma_start(out=xo[:], in_=xo_d[:])

                for it in range(N_CT):
                    y0 = it * CONV_ROWS
                    for oc in range(2):
                        ps = ps1pool.tile([128, CONV_ROWS * OW], f32)
                        first = True
                        for k in range(9):
                            kh, kw = k // 3, k % 3
                            if kw == 1:
                                src, ncol = xo, XCO
                                col0 = 0
                            else:
                                src, ncol = xe, XCE
                                col0 = kw // 2
                            rhs = _ap(
                                src[:],
                                (2 * y0 + kh) * ncol + col0,
                                [[XR * ncol, 128], [2 * ncol, CONV_ROWS], [1, OW]],
                            )
                            nc.tensor.matmul(
                                ps[:],
                                w1t[:, k, oc * 128 : (oc + 1) * 128],
                                rhs,
                                start=first,
                                stop=(k == 8),
                            )
                            first = False
                        # evac: SiLU(psum + t1) -> bf16 interior of h_pad
                        dst = _ap(
                            h_pad[oc][:],
                            (2 + y0) * HP + 2,
                            [[HP * HP, 128], [HP, CONV_ROWS], [1, OW]],
                        )
                        nc.scalar.activation(
                            out=dst,
                            in_=ps[:].rearrange(
                                "p (a b) -> p a b", a=CONV_ROWS
                            ),
                            func=mybir.ActivationFunctionType.Silu,
                            bias=t1[:, oc : oc + 1],
                            scale=1.0,
                        )

            # ---------------- Stage 2+3: offset conv, tents ----------------
            with tc.tile_pool(name="tkeep", bufs=1) as tkeep:
              # tents: T[81, P] bf16; row r = 9k + 3u' + v'  (filled below)
              t_bf = tkeep.tile([81, P], bf16)
              with (
                tc.tile_pool(name="tpool", bufs=1) as tpool,
                tc.tile_pool(name="ps2", bufs=4, space="PSUM") as ps2pool,
              ):
                off_sb = tpool.tile([18, P], f32)
                for it in range(N_CT):
                    y0 = it * CONV_ROWS
                    ps = ps2pool.tile([18, CONV_ROWS * OW], f32)
                    first = True
                    for k in range(9):
                        kh, kw = k // 3, k % 3
                        for cc in range(2):
                            rhs = _ap(
                                h_pad[cc][:],
                                (2 + y0 + kh - 1) * HP + (1 + kw),
                                [[HP * HP, 128], [HP, CONV_ROWS], [1, OW]],
                            )
                            nc.tensor.matmul(
                                ps[:],
                                offt[:, k, cc, :],
                                rhs,
                                start=first,
                                stop=(k == 8 and cc == 1),
                            )
                            first = False
                    nc.vector.tensor_scalar_add(
                        out=off_sb[:, y0 * OW : y0 * OW + CONV_ROWS * OW],
                        in0=ps[:],
                        scalar1=offb[:, 0:1],
                    )
                nc.sync.dma_start(out=offst_d[:], in_=off_sb[:])
                if debug:
                    nc.sync.dma_start(out=offdbg_d[:], in_=off_sb[:])
                    for cc in range(2):
                        hf = tpool.tile([128, HP * HP], f32)
                        nc.vector.tensor_copy(out=hf[:], in_=h_pad[cc][:].rearrange("p a b -> p (a b)"))
                        nc.sync.dma_start(out=hdbg_d[cc], in_=hf[:])

                HALF = P // 2
                for hh in range(2):
                    c0 = hh * HALF
                    oy = tpool.tile([81, HALF], f32, tag="sc_a")
                    ox = tpool.tile([81, HALF], f32, tag="sc_b")
                    za = tpool.tile([81, HALF], f32, tag="sc_c")
                    zb = tpool.tile([81, HALF], f32, tag="sc_d")
                    for k in range(9):
                        # off_y[k] -> oy rows 9k..9k+9 ; off_x[k] -> ox rows
                        nc.sync.dma_start(
                            out=oy[9 * k : 9 * k + 9, :],
                            in_=_ap(offst_d[:], k * P + c0, [[0, 9], [1, HALF]]),
                        )
                        nc.sync.dma_start(
                            out=ox[9 * k : 9 * k + 9, :],
                            in_=_ap(offst_d[:], (9 + k) * P + c0, [[0, 9], [1, HALF]]),
                        )
                    # za = |oy - u| ; oy := relu(1 - za)  (tent_y)
                    nc.scalar.activation(
                        out=za[:], in_=oy[:],
                        func=mybir.ActivationFunctionType.Abs,
                        bias=buv[:81, 0:1], scale=1.0,
                    )
                    nc.scalar.activation(
                        out=oy[:], in_=za[:],
                        func=mybir.ActivationFunctionType.Relu,
                        bias=1.0, scale=-1.0,
                    )
                    nc.scalar.activation(
                        out=zb[:], in_=ox[:],
                        func=mybir.ActivationFunctionType.Abs,
                        bias=buv[:81, 1:2], scale=1.0,
                    )
                    nc.scalar.activation(
                        out=ox[:], in_=zb[:],
                        func=mybir.ActivationFunctionType.Relu,
                        bias=1.0, scale=-1.0,
                    )
                    nc.vector.tensor_mul(t_bf[:, c0 : c0 + HALF], oy[:], ox[:])
                    nc.sync.dma_start(
                        out=tbf_d[:, c0 : c0 + HALF], in_=t_bf[:, c0 : c0 + HALF]
                    )

              # ------------- Stage 4: combine + dconv + BN2 + SiLU -------------
              if True:
                with (
                    tc.tile_pool(name="slab", bufs=2) as spool,
                    tc.tile_pool(name="samp", bufs=1) as smpool,
                    tc.tile_pool(name="ps3", bufs=4, space="PSUM") as ps3pool,
                ):
                    for s in range(N_SLABS):
                        y0 = s * SLAB_ROWS
                        c0 = y0 * OW
                        samp = smpool.tile([128, 9, 2, SL], bf16, tag="samp")
                        for k in range(9):
                            kh, kw = k // 3, k % 3
                            tb = spool.tile([128, 9, SL], bf16, tag="tb")
                            for j in range(9):
                                nc.sync.dma_start(
                                    out=tb[:, j, :],
                                    in_=_ap(
                                        tbf_d[:], (9 * k + j) * P + c0,
                                        [[0, 128], [1, SL]],
                                    ),
                                )
                            for cc in range(2):
                                acc = samp[:, k, cc, :]
                                tmp = spool.tile([128, SL], bf16, tag="tmp")
                                for j in range(9):
                                    u, v = j // 3 - 1, j % 3 - 1
                                    hs = _ap(
                                        h_pad[cc][:],
                                        (2 + y0 + kh - 1 + u) * HP + (2 + kw - 1 + v),
                                        [[HP * HP, 128], [HP, SLAB_ROWS], [1, OW]],
                                    )
                                    tbj = tb[:, j, :].rearrange(
                                        "p (a b) -> p a b", a=SLAB_ROWS
                                    )
                                    if j == 0:
                                        nc.vector.tensor_mul(
                                            acc.rearrange("p (a b) -> p a b", a=SLAB_ROWS),
                                            hs, tbj,
                                        )
                                    else:
                                        nc.vector.tensor_mul(
                                            tmp[:].rearrange("p (a b) -> p a b", a=SLAB_ROWS),
                                            hs, tbj,
                                        )
                                        nc.vector.tensor_add(acc, acc, tmp[:])
                        # dconv matmuls for this slab
                        for oc in range(2):
                            for nt in range(SL // N_MM):
                                ps = ps3pool.tile([128, N_MM], f32)
                                first = True
                                for k in range(9):
                                    for cc in range(2):
                                        nc.tensor.matmul(
                                            ps[:],
                                            dwt[:, k, cc, oc * 128 : (oc + 1) * 128],
                                            samp[:, k, cc, nt * N_MM : (nt + 1) * N_MM],
                                            start=first,
                                            stop=(k == 8 and cc == 1),
                                        )
                                        first = False
                                osb = spool.tile([128, N_MM], f32, tag="osb")
                                nc.scalar.activation(
                                    out=osb[:], in_=ps[:],
                                    func=mybir.ActivationFunctionType.Silu,
                                    bias=t2[:, oc : oc + 1], scale=1.0,
                                )
                                nc.sync.dma_start(
                                    out=out_d[
                                        oc * 128 : (oc + 1) * 128,
                                        c0 + nt * N_MM : c0 + (nt + 1) * N_MM,
                                    ],
                                    in_=osb[:],
                                )
    return nc


def _split_waits_json(raw):
    """This walrus build caps sync waits at 1 per instruction (2 for
    EventSemaphore). Split excess waits onto preceding same-engine NoOps."""
    import json as _json

    m = _json.loads(raw)
    ctr = 0
    for f in m["functions"]:
        for bb in f["blocks"]:
            new_insts = []
            for inst in bb["instructions"]:
                si = inst.get("sync_info")
                waits = (si or {}).get("on_wait") or []
                cap = 2 if inst.get("opcode") == "EventSemaphore" else 1
                if len(waits) > cap:
                    for w in waits[:-cap]:
                        ctr += 1
                        new_insts.append({
                            "debug": inst.get("debug", 0),
                            "engine": inst["engine"],
                            "ins": [], "outs": [],
                            "name": f"WSPLIT-{ctr}",
                            "opcode": "NoOp",
                            "sync_info": {"on_update": [], "on_wait": [w]},
                        })
                    si["on_wait"] = waits[-cap:]
                new_insts.append(inst)
            bb["instructions"] = new_insts
    return _json.dumps(m).encode()


_CACHE = {}


def _get_nc(debug=False):
    if debug not in _CACHE:
        nc = _build(debug)
        patched = _split_waits_json(nc.to_json_bytes())
        nc.to_json_bytes = lambda p=patched: p
        _CACHE[debug] = nc
    return _CACHE[debug]


def _prep_inputs(x, conv1_w, bn1_g, bn1_b, bn1_m, bn1_v, off_w, off_b, dconv_w,
                 bn2_g, bn2_b, bn2_m, bn2_v):
    x = np.asarray(x, np.float32)
    # padded, even/odd split input
    xp = np.zeros((B, C1, XR, XR), np.float32)
    xp[:, :, 1:, 1:] = x
    xe = np.ascontiguousarray(xp[:, :, :, 0::2])  # [B,128,161,81]
    xo = np.ascontiguousarray(xp[:, :, :, 1::2])  # [B,128,161,80]

    s1 = np.asarray(bn1_g, np.float32) / np.sqrt(np.asarray(bn1_v, np.float32) + EPS)
    w1f = np.asarray(conv1_w, np.float32) * s1[:, None, None, None]
    t1 = np.asarray(bn1_b, np.float32) - np.asarray(bn1_m, np.float32) * s1
    # w1t[c, k, o]
    w1t = np.ascontiguousarray(
        w1f.reshape(C2, C1, 9).transpose(1, 2, 0)
    ).astype(np.float32)

    # offset conv: output channel j: j<9 -> orig 2j (y), j>=9 -> 2(j-9)+1 (x)
    chmap = [2 * j for j in range(9)] + [2 * j + 1 for j in range(9)]
    offw = np.asarray(off_w, np.float32)[chmap]  # [18, 256, 3, 3]
    offbp = np.asarray(off_b, np.float32)[chmap].reshape(18, 1)
    # offt[c_within, k, cc, j]
    offt = np.ascontiguousarray(
        offw.reshape(18, 2, 128, 9).transpose(2, 3, 1, 0)
    ).astype(ml_dtypes.bfloat16)

    s2 = np.asarray(bn2_g, np.float32) / np.sqrt(np.asarray(bn2_v, np.float32) + EPS)
    dwf = np.asarray(dconv_w, np.float32) * s2[:, None, None, None]
    t2 = np.asarray(bn2_b, np.float32) - np.asarray(bn2_m, np.float32) * s2
    # dwt[c_within, k, cc, o]
    dwt = np.ascontiguousarray(
        dwf.reshape(C2, 2, 128, 9).transpose(2, 3, 1, 0)
    ).astype(ml_dtypes.bfloat16)

    t1s = np.ascontiguousarray(t1.reshape(2, 128).T).astype(np.float32)
    t2s = np.ascontiguousarray(t2.reshape(2, 128).T).astype(np.float32)

    buv = np.zeros((81, 2), np.float32)
    for r in range(81):
        buv[r, 0] = -((r % 9) // 3 - 1)
        buv[r, 1] = -(r % 3 - 1)

    shared = dict(w1t=w1t, offt=offt, dwt=dwt, t1=t1s, t2=t2s, offb=offbp, buv=buv)
    in_maps = []
    for b in range(B):
        m = dict(shared)
        m["xe"] = np.ascontiguousarray(xe[b])
        m["xo"] = np.ascontiguousarray(xo[b])
        in_maps.append(m)
    return in_maps


def _run(in_maps, debug=False, trace=False):
    from concourse.bass_utils import run_bass_kernel_spmd

    nc = _get_nc(debug)
    return run_bass_kernel_spmd(
        nc, in_maps, core_ids=list(range(N_CORES)), trace=trace
    )


def kernel(**inputs):
    in_maps = _prep_inputs(**inputs)
    res = _run(in_maps, debug=False)
    out = np.stack(
        [r["out"].reshape(C2, OH, OW) for r in res.results]
    ).astype(np.float32)
    return out
